# revision 19
# baseline (speedup 1.0000x reference)
"""Distributed Bass kernel: attention with distance-based positional weights + LayerNorm.

nn_Attention: B=2, S=2048, E=1024, H=16 (d=64), fp32.
  q/k/v = x @ W{q,k,v}.T ; S = q.k^T * E**-0.5 * (|i-j|/S) ; P = softmax(S)
  out = LayerNorm(P @ v)

Sharding: tensor-parallel over heads. 8 cores x 2 heads (128 features each).

v2 schedule: proj(b0) -> attention(b0) with proj(b1) interleaved as 12
sub-chunks (keeps PE warm and lets ScalarE exp start ~60us earlier) ->
attention(b1). Attention units are software-pipelined across unit
boundaries (next unit's first two score tiles issue before the previous
unit's normalize epilogue) so ScalarE never starves.

Distance-weight trick: D = (qi-ki)*(q.k) from one 128-contraction matmul
with index-augmented operands (KA = [K^T; k*K^T], QA = [q*Q^T; -Q^T]).
exp(c*sign(qi-ki)*D) is handled with a single ACT call per [128,1024]
tile: a negated copy of KA (built by the otherwise-idle GpSimd engine)
serves the qi<ki region so both regions use exp(+c * pd); the 128x128
diagonal block is fixed by a DVE sign-mask multiply pre-exp.

LayerNorm rstd uses a DVE bit-trick rsqrt + 2 Newton steps (no Ln/Exp
table switching on ScalarE - the baseline reloaded ACT tables 9x).
"""

import sys
import numpy as np

for _p in ("/opt/trn_rl_repo", "/root/.axon_site/_ro/trn_rl_repo"):
    if _p not in sys.path:
        sys.path.append(_p)

from concourse import bass, bacc, tile, mybir  # noqa: E402
from concourse import bass_utils  # noqa: E402

dt = mybir.dt
AF = mybir.ActivationFunctionType
ALU = mybir.AluOpType

B, S, E, H = 2, 2048, 1024, 16
D = E // H                      # 64
NCORES = 8
PF = E // NCORES                # 128 features per core (2 heads)
NT = B * S                      # 4096 tokens
HTOK = 128                      # tokens per core per (batch, stripe) for LN
NKT = S // 128                  # 16 k-tiles per batch
NET = E // 128                  # 8 e-tiles (contraction) per projection
STRIPE = 1024                   # q-stripe width in attention inner loop
NST = S // STRIPE               # 2 stripes per batch
QC = 512                        # proj token-group width
NQC = S // QC                   # 4 proj groups per batch
CEXP = 1.0 / (32.0 * 2048.0)    # E**-0.5 / S
EPS = 1e-5
RSQRT_MAGIC = 0x5F3759DF

F32R = dt.float32r
F32 = dt.float32
BF16 = dt.bfloat16

_CACHE = {}


def _build(ln_trivial):
    nc = bacc.Bacc("TRN2", target_bir_lowering=False, debug=False,
                   num_devices=NCORES)

    xT = nc.dram_tensor("xT", [E, NT], BF16, kind="ExternalInput").ap()
    wq = nc.dram_tensor("wq", [E, PF], BF16, kind="ExternalInput").ap()
    wk = nc.dram_tensor("wk", [E, PF], BF16, kind="ExternalInput").ap()
    wv = nc.dram_tensor("wv", [E, PF], BF16, kind="ExternalInput").ap()
    # consts packed as [128, 2048 qidx | 128 sgnc | 1 ones]
    cst_d = nc.dram_tensor("cst", [128, S + 129], F32R,
                           kind="ExternalInput").ap()
    onesr_d = nc.dram_tensor("onesr", [1, 128], F32, kind="ExternalInput").ap()
    identb_d = nc.dram_tensor("identb", [128, 128], BF16,
                              kind="ExternalInput").ap()
    gb_d = nc.dram_tensor("gb", [128, 2 * NET], F32, kind="ExternalInput").ap()
    out_d = nc.dram_tensor("out", [E, 4 * HTOK], F32R,
                           kind="ExternalOutput").ap()

    with tile.TileContext(nc) as tc:
        with (
            tc.tile_pool(name="res", bufs=1) as res,
            tc.tile_pool(name="work", bufs=1) as work,
            tc.tile_pool(name="psum", bufs=1, space="PSUM") as psum,
            tc.tile_pool(name="dram", bufs=1, space="DRAM") as dram,
            nc.allow_low_precision(reason="float32r is fp32 storage"),
        ):
            # ---------- resident constants ----------
            wq_sb = res.tile([128, NET * 128], BF16, name="wq_sb")
            wk_sb = res.tile([128, NET * 128], BF16, name="wk_sb")
            wv_sb = res.tile([128, NET * 128], BF16, name="wv_sb")
            # spread weight loads over two queues: each strided load costs
            # ~3.5us of descriptor generation, which serialized startup
            for w_sb, w_d, eng in ((wq_sb, wq, nc.scalar),
                                   (wk_sb, wk, nc.gpsimd),
                                   (wv_sb, wv, nc.scalar)):
                eng.dma_start(
                    w_sb[:].rearrange("p (a j) -> p a j", j=128),
                    w_d.rearrange("(a p) j -> p a j", p=128))
            cst = res.tile([128, S + 129], F32R, name="cst")
            qidx = cst[:, 0:S]
            sgnc = cst[:, S:S + 128]
            ones_col = cst[:, S + 128:S + 129]
            ones_row = res.tile([1, 128], F32, name="ones_row")
            identb = res.tile([128, 128], BF16, name="identb")
            gb_sb = None
            if not ln_trivial:
                gb_sb = res.tile([128, 2 * NET], F32, name="gb_sb")

            def load_consts():
                # issued after the first two xt chunks so the sync queue
                # delivers the data the first matmuls need first
                nc.sync.dma_start(cst[:], cst_d[:])
                nc.sync.dma_start(ones_row[:], onesr_d[:])
                nc.sync.dma_start(identb[:], identb_d[:])
                if not ln_trivial:
                    nc.sync.dma_start(gb_sb[:], gb_d[:])

            a2a_in = [dram.tile([NCORES * 128, HTOK], F32R, name=f"a2a_in{q}")
                      for q in range(4)]
            a2a_out = [dram.tile([NCORES * 128, HTOK], F32R,
                                 name=f"a2a_out{q}")
                       for q in range(4)]

            # ---------- persistent per-(b,h) tiles ----------
            qa, ka, kan, vsb = {}, {}, {}, {}
            for b in range(B):
                for h in range(2):
                    qa[b, h] = work.tile([128, S], BF16, tag=f"qa{b}{h}",
                                         name=f"qa{b}{h}")
                    ka[b, h] = work.tile([128, S], BF16, tag=f"ka{b}{h}",
                                         name=f"ka{b}{h}")
                    kan[b, h] = work.tile([128, S], BF16, tag=f"kan{b}{h}",
                                          name=f"kan{b}{h}")
                    vsb[b, h] = work.tile([128, NKT * 65], BF16,
                                          tag=f"v{b}{h}", name=f"v{b}{h}")
                    # only the softmax-denominator ones-column needs init
                    nc.vector.memset(
                        vsb[b, h][:].rearrange("p (k c) -> p k c", c=65)[
                            :, :, 64:65], 1.0)
            vt = {b: work.tile([128, S], BF16, tag="vt", name=f"vt{b}")
                  for b in range(B)}
            outT = {b: work.tile([128, S], F32R, tag=f"outT{b}",
                                 name=f"outT{b}") for b in range(B)}

            # ---------- projections ----------
            xt_tiles = {}

            def xt_load(b, qc):
                if (b, qc) in xt_tiles:
                    return xt_tiles[b, qc]
                xt_c = work.tile([128, NET * QC], BF16, tag="xt", bufs=3,
                                 name=f"xt{b}{qc}")
                nc.sync.dma_start(
                    xt_c[:].rearrange("p (a n) -> p a n", n=QC),
                    xT.rearrange("(a p) n -> p a n", p=128)[
                        :, :, b * S + qc * QC:b * S + (qc + 1) * QC])
                xt_tiles[b, qc] = xt_c
                return xt_c

            def proj_part(b, qc, which):
                """One 512-token projection group for one of q/k/v."""
                xt_c = xt_load(b, qc)
                gsl = slice(qc * QC, (qc + 1) * QC)
                w_sb = {"q": wq_sb, "k": wk_sb, "v": wv_sb}[which]
                pp = psum.tile([128, QC], F32, tag="d", bufs=2,
                               name=f"p{which}{b}{qc}")
                for kt in range(NET):
                    nc.tensor.matmul(pp[:, 0:QC],
                                     w_sb[:, kt * 128:(kt + 1) * 128],
                                     xt_c[:, kt * QC:(kt + 1) * QC],
                                     start=(kt == 0), stop=(kt == NET - 1))
                if which == "q":
                    for h in range(2):
                        hs = slice(h * 64, h * 64 + 64)
                        nc.vector.tensor_tensor(qa[b, h][0:64, gsl],
                                                pp[hs, 0:QC],
                                                qidx[0:64, gsl], ALU.mult)
                        nc.vector.tensor_scalar_mul(qa[b, h][64:128, gsl],
                                                    pp[hs, 0:QC], -1.0)
                elif which == "k":
                    for h in range(2):
                        hs = slice(h * 64, h * 64 + 64)
                        nc.vector.tensor_copy(ka[b, h][0:64, gsl],
                                              pp[hs, 0:QC])
                        nc.vector.tensor_tensor(ka[b, h][64:128, gsl],
                                                pp[hs, 0:QC],
                                                qidx[0:64, gsl], ALU.mult)
                    for h in range(2):
                        # DVE, not GpSimd: a Q7 negate measured 7.4us per
                        # [128,512] tile and clogged the gpsimd FIFO ahead
                        # of the collective triggers
                        nc.vector.tensor_scalar_mul(kan[b, h][:, gsl],
                                                    ka[b, h][:, gsl], -1.0)
                else:
                    nc.vector.tensor_copy(vt[b][:, gsl], pp[:, 0:QC])
                    for c in range(qc * 4, qc * 4 + 4):
                        pt_ps = psum.tile([128, 128], BF16, tag="db", bufs=2,
                                          name=f"ptr{b}{c}")
                        nc.tensor.transpose(pt_ps[:],
                                            vt[b][:, c * 128:(c + 1) * 128],
                                            identb[:])
                        for h in range(2):
                            nc.vector.tensor_copy(
                                vsb[b, h][:, c * 65:c * 65 + 64],
                                pt_ps[:, h * 64:h * 64 + 64])

            # ---------- LayerNorm for one token-quarter ----------
            def layer_norm(q):
                gt = work.tile([128, NET * HTOK], F32R, tag="gt", bufs=2,
                               name=f"gt{q}")
                nc.gpsimd.dma_start(
                    gt[:].rearrange("p (a n) -> p a n", n=HTOK),
                    a2a_out[q][:].rearrange("(a p) n -> p a n", p=128))
                ps_s = psum.tile([1, HTOK], F32, tag="d", bufs=2,
                                 name=f"ps_s{q}")
                ps_q = psum.tile([1, HTOK], F32, tag="d", bufs=2,
                                 name=f"ps_q{q}")
                for kt in range(NET):
                    nc.tensor.matmul(ps_s[:], ones_col,
                                     gt[:, kt * HTOK:(kt + 1) * HTOK],
                                     start=(kt == 0), stop=(kt == NET - 1))
                sq = work.tile([128, 2 * HTOK], F32R, tag="sq", bufs=2,
                               name=f"sq{q}")
                for kt in range(NET):
                    ssl = slice((kt % 2) * HTOK, (kt % 2) * HTOK + HTOK)
                    nc.vector.tensor_tensor(sq[:, ssl],
                                            gt[:, kt * HTOK:(kt + 1) * HTOK],
                                            gt[:, kt * HTOK:(kt + 1) * HTOK],
                                            ALU.mult)
                    nc.tensor.matmul(ps_q[:], ones_col, sq[:, ssl],
                                     start=(kt == 0), stop=(kt == NET - 1))
                # scalar row: [mn | m2 | var | t | rstd | nmr]
                sc = work.tile([1, 6 * HTOK], F32, tag="lns", bufs=2,
                               name=f"lns{q}")
                mn = sc[:, 0:HTOK]
                m2 = sc[:, HTOK:2 * HTOK]
                var = sc[:, 2 * HTOK:3 * HTOK]
                tq = sc[:, 3 * HTOK:4 * HTOK]
                rstd = sc[:, 4 * HTOK:5 * HTOK]
                nmr = sc[:, 5 * HTOK:6 * HTOK]
                nc.vector.tensor_scalar_mul(mn, ps_s[:], -1.0 / E)
                nc.vector.tensor_tensor(m2, mn, mn, ALU.mult)
                nc.vector.tensor_scalar(var, ps_q[:], 1.0 / E, EPS,
                                        ALU.mult, ALU.add)
                nc.vector.tensor_tensor(var, var, m2, ALU.subtract)
                # rstd = rsqrt(var): bit-trick seed + 2 Newton steps (pure
                # DVE; avoids the Ln/Exp ACT-table reload the ScalarE path
                # pays every call)
                iv = var.bitcast(dt.int32)
                irs = rstd.bitcast(dt.int32)
                nc.vector.tensor_single_scalar(irs, iv, 1,
                                               ALU.arith_shift_right)
                nc.vector.tensor_scalar(irs, irs, -1, RSQRT_MAGIC,
                                        ALU.mult, ALU.add)
                for _ in range(2):
                    nc.vector.tensor_tensor(tq, rstd, rstd, ALU.mult)
                    nc.vector.tensor_tensor(tq, tq, var, ALU.mult)
                    nc.vector.tensor_scalar(tq, tq, -0.5, 1.5,
                                            ALU.mult, ALU.add)
                    nc.vector.tensor_tensor(rstd, rstd, tq, ALU.mult)
                nc.vector.tensor_tensor(nmr, mn, rstd, ALU.mult)
                pa = psum.tile([128, HTOK], F32, tag="d", bufs=2,
                               name=f"pa{q}")
                pb = psum.tile([128, HTOK], F32, tag="d", bufs=2,
                               name=f"pb{q}")
                nc.tensor.matmul(pa[:], ones_row[:], rstd,
                                 start=True, stop=True)
                nc.tensor.matmul(pb[:], ones_row[:], nmr,
                                 start=True, stop=True)
                t1 = work.tile([128, NET * HTOK], F32R, tag="t1", bufs=2,
                               name=f"t1{q}")
                for kt in range(NET):
                    tsl = slice(kt * HTOK, (kt + 1) * HTOK)
                    nc.vector.tensor_tensor(t1[:, tsl],
                                            gt[:, kt * HTOK:(kt + 1) * HTOK],
                                            pa[:], ALU.mult)
                    nc.vector.tensor_tensor(t1[:, tsl], t1[:, tsl], pb[:],
                                            ALU.add)
                    if not ln_trivial:
                        nc.vector.tensor_scalar(t1[:, tsl], t1[:, tsl],
                                                gb_sb[:, kt:kt + 1],
                                                gb_sb[:, NET + kt:
                                                      NET + kt + 1],
                                                ALU.mult, ALU.add)
                nc.sync.dma_start(
                    out_d.rearrange("(a p) n -> p a n", p=128)[
                        :, :, q * HTOK:(q + 1) * HTOK],
                    t1[:].rearrange("p (a n) -> p a n", n=HTOK))

            # ---------- attention ----------
            pd_tiles = {}

            def issue_score(b, st, h, kt):
                """Score matmuls for one k-tile + diag fixup + exp."""
                pd = psum.tile([128, STRIPE], F32, tag="db", bufs=2,
                               name=f"pd{b}{st}{kt}{h}")
                pd_tiles[b, st, h, kt] = pd
                bound = min(max((kt + 1) * 128 - st * STRIPE, 0), STRIPE)
                for g2 in range(STRIPE // 512):
                    lo, hi = g2 * 512, g2 * 512 + 512
                    # [lo, min(hi,bound)): qi < ki -> negated KA so the
                    # whole tile uses exp(+c * pd)
                    if bound > lo:
                        e = min(hi, bound)
                        nc.tensor.matmul(
                            pd[:, lo:e],
                            kan[b, h][:, kt * 128:(kt + 1) * 128],
                            qa[b, h][:, st * STRIPE + lo:st * STRIPE + e],
                            start=True, stop=True)
                    if bound < hi:
                        s0 = max(lo, bound)
                        nc.tensor.matmul(
                            pd[:, s0:hi],
                            ka[b, h][:, kt * 128:(kt + 1) * 128],
                            qa[b, h][:, st * STRIPE + s0:st * STRIPE + hi],
                            start=True, stop=True)
                if kt * 128 >= st * STRIPE and \
                   (kt + 1) * 128 <= (st + 1) * STRIPE:
                    dl = kt * 128 - st * STRIPE
                    nc.vector.tensor_tensor(
                        pd[:, dl:dl + 128], pd[:, dl:dl + 128],
                        sgnc, ALU.mult)

            def issue_exp(b, st, h, kt):
                pd = pd_tiles[b, st, h, kt]
                ptile = work.tile([128, STRIPE], BF16, tag="pt", bufs=6,
                                  name=f"pt{b}{st}{kt}{h}")
                nc.scalar.activation(ptile[:], pd[:], AF.Exp, scale=CEXP)
                return ptile

            units = [(b, st, h) for b in range(B) for st in range(NST)
                     for h in range(2)]

            def make_tail(b, st, h, po):
                def tail():
                    # normalize rows 0:64 by row 64 (softmax denominator)
                    den = work.tile([1, 2 * STRIPE], F32, tag="den",
                                    bufs=2, name=f"den{b}{st}{h}")
                    nc.vector.tensor_copy(den[:, 0:STRIPE], po[64:65, :])
                    nc.vector.reciprocal_approx_fast(
                        den[:, STRIPE:], den[:, 0:STRIPE])
                    bc_sb = work.tile([64, STRIPE], F32R, tag="bcsb",
                                      bufs=2, name=f"bc{b}{st}{h}")
                    for g2 in range(2):
                        pbc = psum.tile([64, 512], F32, tag="d", bufs=2,
                                        name=f"pbc{b}{st}{h}{g2}")
                        nc.tensor.matmul(
                            pbc[:], ones_row[:, 0:64],
                            den[:, STRIPE + g2 * 512:
                                STRIPE + (g2 + 1) * 512],
                            start=True, stop=True)
                        nc.vector.tensor_copy(
                            bc_sb[:, g2 * 512:(g2 + 1) * 512], pbc[:])
                    nc.vector.tensor_tensor(
                        outT[b][h * 64:(h + 1) * 64,
                                st * STRIPE:(st + 1) * STRIPE],
                        po[0:64, :], bc_sb[:], ALU.mult)
                    if h == 1:
                        # stripe complete: per-stripe AllToAll re-shards
                        # feature-blocks -> token-blocks for LayerNorm
                        q = b * NST + st
                        nc.sync.dma_start(
                            a2a_in[q][:].rearrange("(j p) n -> p j n", p=128),
                            outT[b][:, st * STRIPE:(st + 1) * STRIPE]
                            .rearrange("p (j n) -> p j n", n=HTOK))
                        nc.gpsimd.collective_compute(
                            "AllToAll", ALU.bypass,
                            replica_groups=[list(range(NCORES))],
                            ins=[a2a_in[q].opt()], outs=[a2a_out[q].opt()])
                        # layer_norm(q) is NOT issued here: its PE matmuls
                        # would head-of-line block the PE FIFO on the
                        # collective (observed 18us stall); deferred to
                        # ~2 units later in the main loop
                return tail

            # b0 projections (ScalarE is necessarily idle here; keep PE+DVE
            # dense). Part order q,v,k: v's matmuls reuse q's psum ring
            # slot only after q's 2-op epilogue (vs k's 6-op one), so the
            # PE never waits on the DVE queue between parts.
            xt_load(0, 0)
            xt_load(0, 1)
            load_consts()
            for qc in range(NQC):
                if qc + 2 < NQC:
                    xt_load(0, qc + 2)
                for which in ("q", "v", "k"):
                    proj_part(0, qc, which)

            # prefetch b1's first token group before attention starts
            xt_load(1, 0)

            # b1 projection work, fed into attention(b0)'s PE slack
            thunks = []
            for qc in range(NQC):
                for which in ("q", "v", "k"):
                    def mk(qc=qc, which=which):
                        def run():
                            if which == "q" and qc + 1 < NQC:
                                xt_load(1, qc + 1)
                            proj_part(1, qc, which)
                        return run
                    thunks.append(mk())
            thunks.reverse()  # pop() from the front

            prev_tail = None
            for ui, (b, st, h) in enumerate(units):
                issue_score(b, st, h, 0)
                issue_score(b, st, h, 1)
                if prev_tail is not None:
                    prev_tail()
                # deferred LayerNorms: far enough after their AllToAll
                # that the collective (rendezvous + transfer, ~70us for
                # the first one) is done before the PE reaches them
                if ui == 6:
                    layer_norm(0)
                elif ui == 7:
                    layer_norm(1)
                po = psum.tile([65, STRIPE], F32, tag="o",
                               name=f"po{h}_{b}{st}")
                for kt in range(NKT):
                    ptile = issue_exp(b, st, h, kt)
                    if kt + 2 < NKT:
                        issue_score(b, st, h, kt + 2)
                    for g2 in range(STRIPE // 512):
                        nc.tensor.matmul(
                            po[:, g2 * 512:(g2 + 1) * 512],
                            vsb[b, h][:, kt * 65:(kt + 1) * 65],
                            ptile[:, g2 * 512:(g2 + 1) * 512],
                            start=(kt == 0), stop=(kt == NKT - 1))
                    if kt in (3, 7, 11) and thunks:
                        thunks.pop()()
                pd_tiles.clear()
                prev_tail = make_tail(b, st, h, po)
            prev_tail()
            layer_norm(2)
            layer_norm(3)

    nc.compile()
    return nc


def _host_inputs(x, Wq, Wk, Wv, ln_gamma, ln_beta):
    import ml_dtypes
    bf16 = ml_dtypes.bfloat16
    xT = np.ascontiguousarray(
        x.reshape(NT, E).T.astype(np.float32)).astype(bf16)
    cst = np.zeros((128, S + 129), np.float32)
    cst[:, 0:S] = np.arange(S, dtype=np.float32)[None, :]
    jj = np.arange(128, dtype=np.float32)
    cst[:, S:S + 128] = -np.sign(jj[None, :] - jj[:, None])
    cst[:, S + 128] = 1.0
    gb = np.zeros((128, 2 * NET), np.float32)
    gb[:, 0:NET] = np.asarray(ln_gamma, np.float32).reshape(NET, 128).T
    gb[:, NET:] = np.asarray(ln_beta, np.float32).reshape(NET, 128).T
    in_maps = []
    for c in range(NCORES):
        fsl = slice(c * PF, (c + 1) * PF)
        in_maps.append({
            "xT": xT,
            "wq": np.ascontiguousarray(
                np.asarray(Wq, np.float32)[fsl, :].T).astype(bf16),
            "wk": np.ascontiguousarray(
                np.asarray(Wk, np.float32)[fsl, :].T).astype(bf16),
            "wv": np.ascontiguousarray(
                np.asarray(Wv, np.float32)[fsl, :].T).astype(bf16),
            "cst": cst,
            "onesr": np.ones((1, 128), np.float32),
            "identb": np.eye(128).astype(bf16),
            "gb": gb,
        })
    return in_maps


def kernel(x, Wq, Wk, Wv, ln_gamma, ln_beta, _trace=False, _tmpdir=None):
    ln_trivial = bool(np.all(np.asarray(ln_gamma) == 1.0)
                      and np.all(np.asarray(ln_beta) == 0.0))
    key = ("nc", ln_trivial)
    if key not in _CACHE:
        _CACHE[key] = _build(ln_trivial)
    nc = _CACHE[key]
    in_maps = _host_inputs(x, Wq, Wk, Wv, ln_gamma, ln_beta)
    res = bass_utils.run_bass_kernel_spmd(
        nc, in_maps, core_ids=list(range(NCORES)),
        trace=_trace, tmpdir=_tmpdir)
    _CACHE["last_result"] = res
    # out[c] is [E, 4*HTOK]: quarter q=(b*2+st) at cols [q*128,(q+1)*128)
    # = tokens [b*S + st*1024 + 128c, +128).
    outT = np.empty((E, NT), np.float32)
    for c in range(NCORES):
        o = np.asarray(res.results[c]["out"])
        for q in range(4):
            b, st = q // 2, q % 2
            base = b * S + st * STRIPE + c * HTOK
            outT[:, base:base + HTOK] = o[:, q * HTOK:(q + 1) * HTOK]
    return np.ascontiguousarray(outT.T).reshape(B, S, E).astype(np.float32)


# revision 20
# speedup vs baseline: 1.0051x; 1.0051x over previous
"""Distributed Bass kernel: attention with distance-based positional weights + LayerNorm.

nn_Attention: B=2, S=2048, E=1024, H=16 (d=64), fp32.
  q/k/v = x @ W{q,k,v}.T ; S = q.k^T * E**-0.5 * (|i-j|/S) ; P = softmax(S)
  out = LayerNorm(P @ v)

Sharding: tensor-parallel over heads. 8 cores x 2 heads (128 features each).

v2 schedule: proj(b0) -> attention(b0) with proj(b1) interleaved as 12
sub-chunks (keeps PE warm and lets ScalarE exp start ~60us earlier) ->
attention(b1). Attention units are software-pipelined across unit
boundaries (next unit's first two score tiles issue before the previous
unit's normalize epilogue) so ScalarE never starves.

Distance-weight trick: D = (qi-ki)*(q.k) from one 128-contraction matmul
with index-augmented operands (KA = [K^T; k*K^T], QA = [q*Q^T; -Q^T]).
exp(c*sign(qi-ki)*D) is handled with a single ACT call per [128,1024]
tile: a negated copy of KA (built by the otherwise-idle GpSimd engine)
serves the qi<ki region so both regions use exp(+c * pd); the 128x128
diagonal block is fixed by a DVE sign-mask multiply pre-exp.

LayerNorm rstd uses a DVE bit-trick rsqrt + 2 Newton steps (no Ln/Exp
table switching on ScalarE - the baseline reloaded ACT tables 9x).
"""

import sys
import numpy as np

for _p in ("/opt/trn_rl_repo", "/root/.axon_site/_ro/trn_rl_repo"):
    if _p not in sys.path:
        sys.path.append(_p)

from concourse import bass, bacc, tile, mybir  # noqa: E402
from concourse import bass_utils  # noqa: E402

dt = mybir.dt
AF = mybir.ActivationFunctionType
ALU = mybir.AluOpType

B, S, E, H = 2, 2048, 1024, 16
D = E // H                      # 64
NCORES = 8
PF = E // NCORES                # 128 features per core (2 heads)
NT = B * S                      # 4096 tokens
HTOK = 128                      # tokens per core per (batch, stripe) for LN
NKT = S // 128                  # 16 k-tiles per batch
NET = E // 128                  # 8 e-tiles (contraction) per projection
STRIPE = 1024                   # q-stripe width in attention inner loop
NST = S // STRIPE               # 2 stripes per batch
QC = 512                        # proj token-group width
NQC = S // QC                   # 4 proj groups per batch
CEXP = 1.0 / (32.0 * 2048.0)    # E**-0.5 / S
EPS = 1e-5
RSQRT_MAGIC = 0x5F3759DF

F32R = dt.float32r
F32 = dt.float32
BF16 = dt.bfloat16

_CACHE = {}


def _build(ln_trivial):
    nc = bacc.Bacc("TRN2", target_bir_lowering=False, debug=False,
                   num_devices=NCORES)

    xT = nc.dram_tensor("xT", [E, NT], BF16, kind="ExternalInput").ap()
    wq = nc.dram_tensor("wq", [E, PF], BF16, kind="ExternalInput").ap()
    wk = nc.dram_tensor("wk", [E, PF], BF16, kind="ExternalInput").ap()
    wv = nc.dram_tensor("wv", [E, PF], BF16, kind="ExternalInput").ap()
    # consts packed as [128, 2048 qidx | 128 sgnc | 1 ones]
    cst_d = nc.dram_tensor("cst", [128, S + 129], F32R,
                           kind="ExternalInput").ap()
    onesr_d = nc.dram_tensor("onesr", [1, 128], F32, kind="ExternalInput").ap()
    identb_d = nc.dram_tensor("identb", [128, 128], BF16,
                              kind="ExternalInput").ap()
    gb_d = nc.dram_tensor("gb", [128, 2 * NET], F32, kind="ExternalInput").ap()
    out_d = nc.dram_tensor("out", [E, 4 * HTOK], F32R,
                           kind="ExternalOutput").ap()

    with tile.TileContext(nc) as tc:
        with (
            tc.tile_pool(name="res", bufs=1) as res,
            tc.tile_pool(name="work", bufs=1) as work,
            tc.tile_pool(name="psum", bufs=1, space="PSUM") as psum,
            tc.tile_pool(name="dram", bufs=1, space="DRAM") as dram,
            nc.allow_low_precision(reason="float32r is fp32 storage"),
        ):
            # ---------- resident constants ----------
            wq_sb = res.tile([128, NET * 128], BF16, name="wq_sb")
            wk_sb = res.tile([128, NET * 128], BF16, name="wk_sb")
            wv_sb = res.tile([128, NET * 128], BF16, name="wv_sb")
            # spread weight loads over two queues: each strided load costs
            # ~3.5us of descriptor generation, which serialized startup
            for w_sb, w_d, eng in ((wq_sb, wq, nc.scalar),
                                   (wk_sb, wk, nc.gpsimd),
                                   (wv_sb, wv, nc.scalar)):
                eng.dma_start(
                    w_sb[:].rearrange("p (a j) -> p a j", j=128),
                    w_d.rearrange("(a p) j -> p a j", p=128))
            cst = res.tile([128, S + 129], F32R, name="cst")
            qidx = cst[:, 0:S]
            sgnc = cst[:, S:S + 128]
            ones_col = cst[:, S + 128:S + 129]
            ones_row = res.tile([1, 128], F32, name="ones_row")
            identb = res.tile([128, 128], BF16, name="identb")
            gb_sb = None
            if not ln_trivial:
                gb_sb = res.tile([128, 2 * NET], F32, name="gb_sb")

            def load_consts():
                # issued after the first two xt chunks so the sync queue
                # delivers the data the first matmuls need first
                nc.sync.dma_start(cst[:], cst_d[:])
                nc.sync.dma_start(ones_row[:], onesr_d[:])
                nc.sync.dma_start(identb[:], identb_d[:])
                if not ln_trivial:
                    nc.sync.dma_start(gb_sb[:], gb_d[:])

            a2a_in = [dram.tile([NCORES * 128, HTOK], F32R, name=f"a2a_in{q}")
                      for q in range(4)]
            a2a_out = [dram.tile([NCORES * 128, HTOK], F32R,
                                 name=f"a2a_out{q}")
                       for q in range(4)]

            # ---------- persistent per-(b,h) tiles ----------
            qa, ka, kan, vsb = {}, {}, {}, {}
            for b in range(B):
                for h in range(2):
                    qa[b, h] = work.tile([128, S], BF16, tag=f"qa{b}{h}",
                                         name=f"qa{b}{h}")
                    ka[b, h] = work.tile([128, S], BF16, tag=f"ka{b}{h}",
                                         name=f"ka{b}{h}")
                    kan[b, h] = work.tile([128, S], BF16, tag=f"kan{b}{h}",
                                          name=f"kan{b}{h}")
                    vsb[b, h] = work.tile([128, NKT * 65], BF16,
                                          tag=f"v{b}{h}", name=f"v{b}{h}")
                    # only the softmax-denominator ones-column needs init
                    nc.vector.memset(
                        vsb[b, h][:].rearrange("p (k c) -> p k c", c=65)[
                            :, :, 64:65], 1.0)
            vt = {b: work.tile([128, S], BF16, tag="vt", name=f"vt{b}")
                  for b in range(B)}
            outT = {b: work.tile([128, S], F32R, tag=f"outT{b}",
                                 name=f"outT{b}") for b in range(B)}

            # ---------- projections ----------
            xt_tiles = {}

            def xt_load(b, qc):
                if (b, qc) in xt_tiles:
                    return xt_tiles[b, qc]
                xt_c = work.tile([128, NET * QC], BF16, tag="xt", bufs=3,
                                 name=f"xt{b}{qc}")
                nc.sync.dma_start(
                    xt_c[:].rearrange("p (a n) -> p a n", n=QC),
                    xT.rearrange("(a p) n -> p a n", p=128)[
                        :, :, b * S + qc * QC:b * S + (qc + 1) * QC])
                xt_tiles[b, qc] = xt_c
                return xt_c

            def proj_part(b, qc, which):
                """One 512-token projection group for one of q/k/v."""
                xt_c = xt_load(b, qc)
                gsl = slice(qc * QC, (qc + 1) * QC)
                w_sb = {"q": wq_sb, "k": wk_sb, "v": wv_sb}[which]
                pp = psum.tile([128, QC], F32, tag="d", bufs=2,
                               name=f"p{which}{b}{qc}")
                for kt in range(NET):
                    nc.tensor.matmul(pp[:, 0:QC],
                                     w_sb[:, kt * 128:(kt + 1) * 128],
                                     xt_c[:, kt * QC:(kt + 1) * QC],
                                     start=(kt == 0), stop=(kt == NET - 1))
                if which == "q":
                    for h in range(2):
                        hs = slice(h * 64, h * 64 + 64)
                        nc.vector.tensor_tensor(qa[b, h][0:64, gsl],
                                                pp[hs, 0:QC],
                                                qidx[0:64, gsl], ALU.mult)
                        nc.vector.tensor_scalar_mul(qa[b, h][64:128, gsl],
                                                    pp[hs, 0:QC], -1.0)
                elif which == "k":
                    for h in range(2):
                        hs = slice(h * 64, h * 64 + 64)
                        nc.vector.tensor_copy(ka[b, h][0:64, gsl],
                                              pp[hs, 0:QC])
                        nc.vector.tensor_tensor(ka[b, h][64:128, gsl],
                                                pp[hs, 0:QC],
                                                qidx[0:64, gsl], ALU.mult)
                    for h in range(2):
                        # DVE, not GpSimd: a Q7 negate measured 7.4us per
                        # [128,512] tile and clogged the gpsimd FIFO ahead
                        # of the collective triggers
                        nc.vector.tensor_scalar_mul(kan[b, h][:, gsl],
                                                    ka[b, h][:, gsl], -1.0)
                else:
                    nc.vector.tensor_copy(vt[b][:, gsl], pp[:, 0:QC])
                    for c in range(qc * 4, qc * 4 + 4):
                        pt_ps = psum.tile([128, 128], BF16, tag="db", bufs=2,
                                          name=f"ptr{b}{c}")
                        nc.tensor.transpose(pt_ps[:],
                                            vt[b][:, c * 128:(c + 1) * 128],
                                            identb[:])
                        for h in range(2):
                            nc.vector.tensor_copy(
                                vsb[b, h][:, c * 65:c * 65 + 64],
                                pt_ps[:, h * 64:h * 64 + 64])

            # ---------- LayerNorm for one token-quarter ----------
            def layer_norm(q):
                gt = work.tile([128, NET * HTOK], F32R, tag="gt", bufs=2,
                               name=f"gt{q}")
                nc.gpsimd.dma_start(
                    gt[:].rearrange("p (a n) -> p a n", n=HTOK),
                    a2a_out[q][:].rearrange("(a p) n -> p a n", p=128))
                ps_s = psum.tile([1, HTOK], F32, tag="d", bufs=2,
                                 name=f"ps_s{q}")
                ps_q = psum.tile([1, HTOK], F32, tag="d", bufs=2,
                                 name=f"ps_q{q}")
                for kt in range(NET):
                    nc.tensor.matmul(ps_s[:], ones_col,
                                     gt[:, kt * HTOK:(kt + 1) * HTOK],
                                     start=(kt == 0), stop=(kt == NET - 1))
                sq = work.tile([128, 2 * HTOK], F32R, tag="sq", bufs=2,
                               name=f"sq{q}")
                for kt in range(NET):
                    ssl = slice((kt % 2) * HTOK, (kt % 2) * HTOK + HTOK)
                    nc.vector.tensor_tensor(sq[:, ssl],
                                            gt[:, kt * HTOK:(kt + 1) * HTOK],
                                            gt[:, kt * HTOK:(kt + 1) * HTOK],
                                            ALU.mult)
                    nc.tensor.matmul(ps_q[:], ones_col, sq[:, ssl],
                                     start=(kt == 0), stop=(kt == NET - 1))
                # scalar row: [mn | m2 | var | t | rstd | nmr]
                sc = work.tile([1, 6 * HTOK], F32, tag="lns", bufs=2,
                               name=f"lns{q}")
                mn = sc[:, 0:HTOK]
                m2 = sc[:, HTOK:2 * HTOK]
                var = sc[:, 2 * HTOK:3 * HTOK]
                tq = sc[:, 3 * HTOK:4 * HTOK]
                rstd = sc[:, 4 * HTOK:5 * HTOK]
                nmr = sc[:, 5 * HTOK:6 * HTOK]
                nc.vector.tensor_scalar_mul(mn, ps_s[:], -1.0 / E)
                nc.vector.tensor_tensor(m2, mn, mn, ALU.mult)
                nc.vector.tensor_scalar(var, ps_q[:], 1.0 / E, EPS,
                                        ALU.mult, ALU.add)
                nc.vector.tensor_tensor(var, var, m2, ALU.subtract)
                # rstd = rsqrt(var): bit-trick seed + 2 Newton steps (pure
                # DVE; avoids the Ln/Exp ACT-table reload the ScalarE path
                # pays every call)
                iv = var.bitcast(dt.int32)
                irs = rstd.bitcast(dt.int32)
                nc.vector.tensor_single_scalar(irs, iv, 1,
                                               ALU.arith_shift_right)
                nc.vector.tensor_scalar(irs, irs, -1, RSQRT_MAGIC,
                                        ALU.mult, ALU.add)
                for _ in range(2):
                    nc.vector.tensor_tensor(tq, rstd, rstd, ALU.mult)
                    nc.vector.tensor_tensor(tq, tq, var, ALU.mult)
                    nc.vector.tensor_scalar(tq, tq, -0.5, 1.5,
                                            ALU.mult, ALU.add)
                    nc.vector.tensor_tensor(rstd, rstd, tq, ALU.mult)
                nc.vector.tensor_tensor(nmr, mn, rstd, ALU.mult)
                pa = psum.tile([128, HTOK], F32, tag="d", bufs=2,
                               name=f"pa{q}")
                pb = psum.tile([128, HTOK], F32, tag="d", bufs=2,
                               name=f"pb{q}")
                nc.tensor.matmul(pa[:], ones_row[:], rstd,
                                 start=True, stop=True)
                nc.tensor.matmul(pb[:], ones_row[:], nmr,
                                 start=True, stop=True)
                t1 = work.tile([128, NET * HTOK], F32R, tag="t1", bufs=2,
                               name=f"t1{q}")
                for kt in range(NET):
                    tsl = slice(kt * HTOK, (kt + 1) * HTOK)
                    nc.vector.tensor_tensor(t1[:, tsl],
                                            gt[:, kt * HTOK:(kt + 1) * HTOK],
                                            pa[:], ALU.mult)
                    nc.vector.tensor_tensor(t1[:, tsl], t1[:, tsl], pb[:],
                                            ALU.add)
                    if not ln_trivial:
                        nc.vector.tensor_scalar(t1[:, tsl], t1[:, tsl],
                                                gb_sb[:, kt:kt + 1],
                                                gb_sb[:, NET + kt:
                                                      NET + kt + 1],
                                                ALU.mult, ALU.add)
                nc.sync.dma_start(
                    out_d.rearrange("(a p) n -> p a n", p=128)[
                        :, :, q * HTOK:(q + 1) * HTOK],
                    t1[:].rearrange("p (a n) -> p a n", n=HTOK))

            # ---------- attention ----------
            pd_tiles = {}

            def issue_score(b, st, h, kt):
                """Score matmuls for one k-tile + diag fixup + exp."""
                pd = psum.tile([128, STRIPE], F32, tag="db", bufs=2,
                               name=f"pd{b}{st}{kt}{h}")
                pd_tiles[b, st, h, kt] = pd
                bound = min(max((kt + 1) * 128 - st * STRIPE, 0), STRIPE)
                for g2 in range(STRIPE // 512):
                    lo, hi = g2 * 512, g2 * 512 + 512
                    # [lo, min(hi,bound)): qi < ki -> negated KA so the
                    # whole tile uses exp(+c * pd)
                    if bound > lo:
                        e = min(hi, bound)
                        nc.tensor.matmul(
                            pd[:, lo:e],
                            kan[b, h][:, kt * 128:(kt + 1) * 128],
                            qa[b, h][:, st * STRIPE + lo:st * STRIPE + e],
                            start=True, stop=True)
                    if bound < hi:
                        s0 = max(lo, bound)
                        nc.tensor.matmul(
                            pd[:, s0:hi],
                            ka[b, h][:, kt * 128:(kt + 1) * 128],
                            qa[b, h][:, st * STRIPE + s0:st * STRIPE + hi],
                            start=True, stop=True)
                if kt * 128 >= st * STRIPE and \
                   (kt + 1) * 128 <= (st + 1) * STRIPE:
                    dl = kt * 128 - st * STRIPE
                    nc.vector.tensor_tensor(
                        pd[:, dl:dl + 128], pd[:, dl:dl + 128],
                        sgnc, ALU.mult)

            def issue_exp(b, st, h, kt):
                pd = pd_tiles[b, st, h, kt]
                ptile = work.tile([128, STRIPE], BF16, tag="pt", bufs=6,
                                  name=f"pt{b}{st}{kt}{h}")
                nc.scalar.activation(ptile[:], pd[:], AF.Exp, scale=CEXP)
                return ptile

            units = [(b, st, h) for b in range(B) for st in range(NST)
                     for h in range(2)]

            def make_tail(b, st, h, po):
                def tail():
                    # normalize rows 0:64 by row 64 (softmax denominator)
                    den = work.tile([1, 2 * STRIPE], F32, tag="den",
                                    bufs=2, name=f"den{b}{st}{h}")
                    nc.vector.tensor_copy(den[:, 0:STRIPE], po[64:65, :])
                    nc.vector.reciprocal_approx_fast(
                        den[:, STRIPE:], den[:, 0:STRIPE])
                    bc_sb = work.tile([64, STRIPE], F32R, tag="bcsb",
                                      bufs=2, name=f"bc{b}{st}{h}")
                    for g2 in range(2):
                        pbc = psum.tile([64, 512], F32, tag="d", bufs=2,
                                        name=f"pbc{b}{st}{h}{g2}")
                        nc.tensor.matmul(
                            pbc[:], ones_row[:, 0:64],
                            den[:, STRIPE + g2 * 512:
                                STRIPE + (g2 + 1) * 512],
                            start=True, stop=True)
                        nc.vector.tensor_copy(
                            bc_sb[:, g2 * 512:(g2 + 1) * 512], pbc[:])
                    nc.vector.tensor_tensor(
                        outT[b][h * 64:(h + 1) * 64,
                                st * STRIPE:(st + 1) * STRIPE],
                        po[0:64, :], bc_sb[:], ALU.mult)
                    if h == 1:
                        # stripe complete: per-stripe AllToAll re-shards
                        # feature-blocks -> token-blocks for LayerNorm
                        q = b * NST + st
                        nc.sync.dma_start(
                            a2a_in[q][:].rearrange("(j p) n -> p j n", p=128),
                            outT[b][:, st * STRIPE:(st + 1) * STRIPE]
                            .rearrange("p (j n) -> p j n", n=HTOK))
                        nc.gpsimd.collective_compute(
                            "AllToAll", ALU.bypass,
                            replica_groups=[list(range(NCORES))],
                            ins=[a2a_in[q].opt()], outs=[a2a_out[q].opt()])
                        # layer_norm(q) is NOT issued here: its PE matmuls
                        # would head-of-line block the PE FIFO on the
                        # collective (observed 18us stall); deferred to
                        # ~2 units later in the main loop
                return tail

            # b0 projections (ScalarE is necessarily idle here; keep PE+DVE
            # dense). Part order q,v,k: v's matmuls reuse q's psum ring
            # slot only after q's 2-op epilogue (vs k's 6-op one), so the
            # PE never waits on the DVE queue between parts.
            xt_load(0, 0)
            xt_load(0, 1)
            load_consts()
            for qc in range(NQC):
                if qc + 2 < NQC:
                    xt_load(0, qc + 2)
                for which in ("q", "v", "k"):
                    proj_part(0, qc, which)

            # prefetch b1's first token group before attention starts
            xt_load(1, 0)

            # b1 projection work, fed into attention(b0)'s PE slack
            thunks = []
            for qc in range(NQC):
                for which in ("q", "v", "k"):
                    def mk(qc=qc, which=which):
                        def run():
                            if which == "q" and qc + 1 < NQC:
                                xt_load(1, qc + 1)
                            proj_part(1, qc, which)
                        return run
                    thunks.append(mk())
            thunks.reverse()  # pop() from the front

            prev_tail = None
            for ui, (b, st, h) in enumerate(units):
                issue_score(b, st, h, 0)
                issue_score(b, st, h, 1)
                if prev_tail is not None:
                    prev_tail()
                # deferred LayerNorms: far enough after their AllToAll
                # that the collective (rendezvous + transfer, ~70us for
                # the first one) is done before the PE reaches them
                if ui == 5:
                    layer_norm(0)
                elif ui == 7:
                    layer_norm(1)
                po = psum.tile([65, STRIPE], F32, tag="o",
                               name=f"po{h}_{b}{st}")
                for kt in range(NKT):
                    ptile = issue_exp(b, st, h, kt)
                    if kt + 2 < NKT:
                        issue_score(b, st, h, kt + 2)
                    for g2 in range(STRIPE // 512):
                        nc.tensor.matmul(
                            po[:, g2 * 512:(g2 + 1) * 512],
                            vsb[b, h][:, kt * 65:(kt + 1) * 65],
                            ptile[:, g2 * 512:(g2 + 1) * 512],
                            start=(kt == 0), stop=(kt == NKT - 1))
                    if kt in (3, 7, 11) and thunks:
                        thunks.pop()()
                pd_tiles.clear()
                prev_tail = make_tail(b, st, h, po)
            prev_tail()
            layer_norm(2)
            layer_norm(3)

    nc.compile()
    return nc


def _host_inputs(x, Wq, Wk, Wv, ln_gamma, ln_beta):
    import ml_dtypes
    bf16 = ml_dtypes.bfloat16
    xT = np.ascontiguousarray(
        x.reshape(NT, E).T.astype(np.float32)).astype(bf16)
    cst = np.zeros((128, S + 129), np.float32)
    cst[:, 0:S] = np.arange(S, dtype=np.float32)[None, :]
    jj = np.arange(128, dtype=np.float32)
    cst[:, S:S + 128] = -np.sign(jj[None, :] - jj[:, None])
    cst[:, S + 128] = 1.0
    gb = np.zeros((128, 2 * NET), np.float32)
    gb[:, 0:NET] = np.asarray(ln_gamma, np.float32).reshape(NET, 128).T
    gb[:, NET:] = np.asarray(ln_beta, np.float32).reshape(NET, 128).T
    in_maps = []
    for c in range(NCORES):
        fsl = slice(c * PF, (c + 1) * PF)
        in_maps.append({
            "xT": xT,
            "wq": np.ascontiguousarray(
                np.asarray(Wq, np.float32)[fsl, :].T).astype(bf16),
            "wk": np.ascontiguousarray(
                np.asarray(Wk, np.float32)[fsl, :].T).astype(bf16),
            "wv": np.ascontiguousarray(
                np.asarray(Wv, np.float32)[fsl, :].T).astype(bf16),
            "cst": cst,
            "onesr": np.ones((1, 128), np.float32),
            "identb": np.eye(128).astype(bf16),
            "gb": gb,
        })
    return in_maps


def kernel(x, Wq, Wk, Wv, ln_gamma, ln_beta, _trace=False, _tmpdir=None):
    ln_trivial = bool(np.all(np.asarray(ln_gamma) == 1.0)
                      and np.all(np.asarray(ln_beta) == 0.0))
    key = ("nc", ln_trivial)
    if key not in _CACHE:
        _CACHE[key] = _build(ln_trivial)
    nc = _CACHE[key]
    in_maps = _host_inputs(x, Wq, Wk, Wv, ln_gamma, ln_beta)
    res = bass_utils.run_bass_kernel_spmd(
        nc, in_maps, core_ids=list(range(NCORES)),
        trace=_trace, tmpdir=_tmpdir)
    _CACHE["last_result"] = res
    # out[c] is [E, 4*HTOK]: quarter q=(b*2+st) at cols [q*128,(q+1)*128)
    # = tokens [b*S + st*1024 + 128c, +128).
    outT = np.empty((E, NT), np.float32)
    for c in range(NCORES):
        o = np.asarray(res.results[c]["out"])
        for q in range(4):
            b, st = q // 2, q % 2
            base = b * S + st * STRIPE + c * HTOK
            outT[:, base:base + HTOK] = o[:, q * HTOK:(q + 1) * HTOK]
    return np.ascontiguousarray(outT.T).reshape(B, S, E).astype(np.float32)


# revision 22
# speedup vs baseline: 1.0108x; 1.0056x over previous
"""Distributed Bass kernel: attention with distance-based positional weights + LayerNorm.

nn_Attention: B=2, S=2048, E=1024, H=16 (d=64), fp32.
  q/k/v = x @ W{q,k,v}.T ; S = q.k^T * E**-0.5 * (|i-j|/S) ; P = softmax(S)
  out = LayerNorm(P @ v)

Sharding: tensor-parallel over heads. 8 cores x 2 heads (128 features each).

v2 schedule: proj(b0) -> attention(b0) with proj(b1) interleaved as 12
sub-chunks (keeps PE warm and lets ScalarE exp start ~60us earlier) ->
attention(b1). Attention units are software-pipelined across unit
boundaries (next unit's first two score tiles issue before the previous
unit's normalize epilogue) so ScalarE never starves.

Distance-weight trick: D = (qi-ki)*(q.k) from one 128-contraction matmul
with index-augmented operands (KA = [K^T; k*K^T], QA = [q*Q^T; -Q^T]).
exp(c*sign(qi-ki)*D) is handled with a single ACT call per [128,1024]
tile: a negated copy of KA (built by the otherwise-idle GpSimd engine)
serves the qi<ki region so both regions use exp(+c * pd); the 128x128
diagonal block is fixed by a DVE sign-mask multiply pre-exp.

LayerNorm rstd uses a DVE bit-trick rsqrt + 2 Newton steps (no Ln/Exp
table switching on ScalarE - the baseline reloaded ACT tables 9x).
"""

import sys
import numpy as np

for _p in ("/opt/trn_rl_repo", "/root/.axon_site/_ro/trn_rl_repo"):
    if _p not in sys.path:
        sys.path.append(_p)

from concourse import bass, bacc, tile, mybir  # noqa: E402
from concourse import bass_utils  # noqa: E402

dt = mybir.dt
AF = mybir.ActivationFunctionType
ALU = mybir.AluOpType

B, S, E, H = 2, 2048, 1024, 16
D = E // H                      # 64
NCORES = 8
PF = E // NCORES                # 128 features per core (2 heads)
NT = B * S                      # 4096 tokens
HTOK = 128                      # tokens per core per (batch, stripe) for LN
NKT = S // 128                  # 16 k-tiles per batch
NET = E // 128                  # 8 e-tiles (contraction) per projection
STRIPE = 1024                   # q-stripe width in attention inner loop
NST = S // STRIPE               # 2 stripes per batch
QC = 512                        # proj token-group width
NQC = S // QC                   # 4 proj groups per batch
CEXP = 1.0 / (32.0 * 2048.0)    # E**-0.5 / S
EPS = 1e-5
RSQRT_MAGIC = 0x5F3759DF

F32R = dt.float32r
F32 = dt.float32
BF16 = dt.bfloat16

_CACHE = {}


def _build(ln_trivial):
    nc = bacc.Bacc("TRN2", target_bir_lowering=False, debug=False,
                   num_devices=NCORES)

    xT = nc.dram_tensor("xT", [E, NT], BF16, kind="ExternalInput").ap()
    wq = nc.dram_tensor("wq", [E, PF], BF16, kind="ExternalInput").ap()
    wk = nc.dram_tensor("wk", [E, PF], BF16, kind="ExternalInput").ap()
    wv = nc.dram_tensor("wv", [E, PF], BF16, kind="ExternalInput").ap()
    # consts packed as [128, 2048 qidx | 128 sgnc | 1 ones]
    cst_d = nc.dram_tensor("cst", [128, S + 129], F32R,
                           kind="ExternalInput").ap()
    onesr_d = nc.dram_tensor("onesr", [1, 128], F32, kind="ExternalInput").ap()
    identb_d = nc.dram_tensor("identb", [128, 128], BF16,
                              kind="ExternalInput").ap()
    gb_d = nc.dram_tensor("gb", [128, 2 * NET], F32, kind="ExternalInput").ap()
    out_d = nc.dram_tensor("out", [E, 4 * HTOK], F32R,
                           kind="ExternalOutput").ap()

    with tile.TileContext(nc) as tc:
        with (
            tc.tile_pool(name="res", bufs=1) as res,
            tc.tile_pool(name="work", bufs=1) as work,
            tc.tile_pool(name="psum", bufs=1, space="PSUM") as psum,
            tc.tile_pool(name="dram", bufs=1, space="DRAM") as dram,
            nc.allow_low_precision(reason="float32r is fp32 storage"),
        ):
            # ---------- resident constants ----------
            wq_sb = res.tile([128, NET * 128], BF16, name="wq_sb")
            wk_sb = res.tile([128, NET * 128], BF16, name="wk_sb")
            wv_sb = res.tile([128, NET * 128], BF16, name="wv_sb")
            # spread weight loads over two queues: each strided load costs
            # ~3.5us of descriptor generation, which serialized startup
            for w_sb, w_d, eng in ((wq_sb, wq, nc.scalar),
                                   (wk_sb, wk, nc.gpsimd),
                                   (wv_sb, wv, nc.scalar)):
                eng.dma_start(
                    w_sb[:].rearrange("p (a j) -> p a j", j=128),
                    w_d.rearrange("(a p) j -> p a j", p=128))
            cst = res.tile([128, S + 129], F32R, name="cst")
            qidx = cst[:, 0:S]
            sgnc = cst[:, S:S + 128]
            ones_col = cst[:, S + 128:S + 129]
            ones_row = res.tile([1, 128], F32, name="ones_row")
            identb = res.tile([128, 128], BF16, name="identb")
            gb_sb = None
            if not ln_trivial:
                gb_sb = res.tile([128, 2 * NET], F32, name="gb_sb")

            def load_consts():
                # issued after the first two xt chunks so the sync queue
                # delivers the data the first matmuls need first
                nc.sync.dma_start(cst[:], cst_d[:])
                nc.sync.dma_start(ones_row[:], onesr_d[:])
                nc.sync.dma_start(identb[:], identb_d[:])
                if not ln_trivial:
                    nc.sync.dma_start(gb_sb[:], gb_d[:])

            a2a_in = [dram.tile([NCORES * 128, HTOK], F32R, name=f"a2a_in{q}")
                      for q in range(4)]
            a2a_out = [dram.tile([NCORES * 128, HTOK], F32R,
                                 name=f"a2a_out{q}")
                       for q in range(4)]

            # ---------- persistent per-(b,h) tiles ----------
            qa, ka, kan, vsb = {}, {}, {}, {}
            for b in range(B):
                for h in range(2):
                    qa[b, h] = work.tile([128, S], BF16, tag=f"qa{b}{h}",
                                         name=f"qa{b}{h}")
                    ka[b, h] = work.tile([128, S], BF16, tag=f"ka{b}{h}",
                                         name=f"ka{b}{h}")
                    kan[b, h] = work.tile([128, S], BF16, tag=f"kan{b}{h}",
                                          name=f"kan{b}{h}")
                    vsb[b, h] = work.tile([128, NKT * 65], BF16,
                                          tag=f"v{b}{h}", name=f"v{b}{h}")
                    # only the softmax-denominator ones-column needs init
                    nc.vector.memset(
                        vsb[b, h][:].rearrange("p (k c) -> p k c", c=65)[
                            :, :, 64:65], 1.0)
            vt = {b: work.tile([128, S], BF16, tag="vt", name=f"vt{b}")
                  for b in range(B)}
            outT = {b: work.tile([128, S], F32R, tag=f"outT{b}",
                                 name=f"outT{b}") for b in range(B)}

            # ---------- projections ----------
            xt_tiles = {}

            def xt_load(b, qc):
                if (b, qc) in xt_tiles:
                    return xt_tiles[b, qc]
                xt_c = work.tile([128, NET * QC], BF16, tag="xt", bufs=3,
                                 name=f"xt{b}{qc}")
                nc.sync.dma_start(
                    xt_c[:].rearrange("p (a n) -> p a n", n=QC),
                    xT.rearrange("(a p) n -> p a n", p=128)[
                        :, :, b * S + qc * QC:b * S + (qc + 1) * QC])
                xt_tiles[b, qc] = xt_c
                return xt_c

            def proj_part(b, qc, which):
                """One 512-token projection group for one of q/k/v."""
                xt_c = xt_load(b, qc)
                gsl = slice(qc * QC, (qc + 1) * QC)
                w_sb = {"q": wq_sb, "k": wk_sb, "v": wv_sb}[which]
                pp = psum.tile([128, QC], F32, tag="d", bufs=2,
                               name=f"p{which}{b}{qc}")
                for kt in range(NET):
                    nc.tensor.matmul(pp[:, 0:QC],
                                     w_sb[:, kt * 128:(kt + 1) * 128],
                                     xt_c[:, kt * QC:(kt + 1) * QC],
                                     start=(kt == 0), stop=(kt == NET - 1))
                if which == "q":
                    for h in range(2):
                        hs = slice(h * 64, h * 64 + 64)
                        nc.vector.tensor_tensor(qa[b, h][0:64, gsl],
                                                pp[hs, 0:QC],
                                                qidx[0:64, gsl], ALU.mult)
                        nc.vector.tensor_scalar_mul(qa[b, h][64:128, gsl],
                                                    pp[hs, 0:QC], -1.0)
                elif which == "k":
                    for h in range(2):
                        hs = slice(h * 64, h * 64 + 64)
                        nc.vector.tensor_copy(ka[b, h][0:64, gsl],
                                              pp[hs, 0:QC])
                        nc.vector.tensor_tensor(ka[b, h][64:128, gsl],
                                                pp[hs, 0:QC],
                                                qidx[0:64, gsl], ALU.mult)
                    for h in range(2):
                        # DVE, not GpSimd: a Q7 negate measured 7.4us per
                        # [128,512] tile and clogged the gpsimd FIFO ahead
                        # of the collective triggers
                        nc.vector.tensor_scalar_mul(kan[b, h][:, gsl],
                                                    ka[b, h][:, gsl], -1.0)
                else:
                    nc.vector.tensor_copy(vt[b][:, gsl], pp[:, 0:QC])
                    for c in range(qc * 4, qc * 4 + 4):
                        pt_ps = psum.tile([128, 128], BF16, tag="db", bufs=2,
                                          name=f"ptr{b}{c}")
                        nc.tensor.transpose(pt_ps[:],
                                            vt[b][:, c * 128:(c + 1) * 128],
                                            identb[:])
                        for h in range(2):
                            nc.vector.tensor_copy(
                                vsb[b, h][:, c * 65:c * 65 + 64],
                                pt_ps[:, h * 64:h * 64 + 64])

            # ---------- LayerNorm for one token-quarter ----------
            def layer_norm(q):
                gt = work.tile([128, NET * HTOK], F32R, tag="gt", bufs=2,
                               name=f"gt{q}")
                nc.gpsimd.dma_start(
                    gt[:].rearrange("p (a n) -> p a n", n=HTOK),
                    a2a_out[q][:].rearrange("(a p) n -> p a n", p=128))
                ps_s = psum.tile([1, HTOK], F32, tag="d", bufs=2,
                                 name=f"ps_s{q}")
                ps_q = psum.tile([1, HTOK], F32, tag="d", bufs=2,
                                 name=f"ps_q{q}")
                for kt in range(NET):
                    nc.tensor.matmul(ps_s[:], ones_col,
                                     gt[:, kt * HTOK:(kt + 1) * HTOK],
                                     start=(kt == 0), stop=(kt == NET - 1))
                sq = work.tile([128, 2 * HTOK], F32R, tag="sq", bufs=2,
                               name=f"sq{q}")
                for kt in range(NET):
                    ssl = slice((kt % 2) * HTOK, (kt % 2) * HTOK + HTOK)
                    nc.vector.tensor_tensor(sq[:, ssl],
                                            gt[:, kt * HTOK:(kt + 1) * HTOK],
                                            gt[:, kt * HTOK:(kt + 1) * HTOK],
                                            ALU.mult)
                    nc.tensor.matmul(ps_q[:], ones_col, sq[:, ssl],
                                     start=(kt == 0), stop=(kt == NET - 1))
                # scalar row: [mn | m2 | var | t | rstd | nmr]
                sc = work.tile([1, 6 * HTOK], F32, tag="lns", bufs=2,
                               name=f"lns{q}")
                mn = sc[:, 0:HTOK]
                m2 = sc[:, HTOK:2 * HTOK]
                var = sc[:, 2 * HTOK:3 * HTOK]
                tq = sc[:, 3 * HTOK:4 * HTOK]
                rstd = sc[:, 4 * HTOK:5 * HTOK]
                nmr = sc[:, 5 * HTOK:6 * HTOK]
                nc.vector.tensor_scalar_mul(mn, ps_s[:], -1.0 / E)
                nc.vector.tensor_tensor(m2, mn, mn, ALU.mult)
                nc.vector.tensor_scalar(var, ps_q[:], 1.0 / E, EPS,
                                        ALU.mult, ALU.add)
                nc.vector.tensor_tensor(var, var, m2, ALU.subtract)
                # rstd = rsqrt(var): bit-trick seed + 2 Newton steps (pure
                # DVE; avoids the Ln/Exp ACT-table reload the ScalarE path
                # pays every call)
                iv = var.bitcast(dt.int32)
                irs = rstd.bitcast(dt.int32)
                nc.vector.tensor_single_scalar(irs, iv, 1,
                                               ALU.arith_shift_right)
                nc.vector.tensor_scalar(irs, irs, -1, RSQRT_MAGIC,
                                        ALU.mult, ALU.add)
                for _ in range(2):
                    nc.vector.tensor_tensor(tq, rstd, rstd, ALU.mult)
                    nc.vector.tensor_tensor(tq, tq, var, ALU.mult)
                    nc.vector.tensor_scalar(tq, tq, -0.5, 1.5,
                                            ALU.mult, ALU.add)
                    nc.vector.tensor_tensor(rstd, rstd, tq, ALU.mult)
                nc.vector.tensor_tensor(nmr, mn, rstd, ALU.mult)
                pa = psum.tile([128, HTOK], F32, tag="d", bufs=2,
                               name=f"pa{q}")
                pb = psum.tile([128, HTOK], F32, tag="d", bufs=2,
                               name=f"pb{q}")
                nc.tensor.matmul(pa[:], ones_row[:], rstd,
                                 start=True, stop=True)
                nc.tensor.matmul(pb[:], ones_row[:], nmr,
                                 start=True, stop=True)
                t1 = work.tile([128, NET * HTOK], F32R, tag="t1", bufs=2,
                               name=f"t1{q}")
                for kt in range(NET):
                    tsl = slice(kt * HTOK, (kt + 1) * HTOK)
                    nc.vector.tensor_tensor(t1[:, tsl],
                                            gt[:, kt * HTOK:(kt + 1) * HTOK],
                                            pa[:], ALU.mult)
                    nc.vector.tensor_tensor(t1[:, tsl], t1[:, tsl], pb[:],
                                            ALU.add)
                    if not ln_trivial:
                        nc.vector.tensor_scalar(t1[:, tsl], t1[:, tsl],
                                                gb_sb[:, kt:kt + 1],
                                                gb_sb[:, NET + kt:
                                                      NET + kt + 1],
                                                ALU.mult, ALU.add)
                nc.sync.dma_start(
                    out_d.rearrange("(a p) n -> p a n", p=128)[
                        :, :, q * HTOK:(q + 1) * HTOK],
                    t1[:].rearrange("p (a n) -> p a n", n=HTOK))

            # ---------- attention ----------
            pd_tiles = {}

            def issue_score(b, st, h, kt):
                """Score matmuls for one k-tile + diag fixup + exp."""
                pd = psum.tile([128, STRIPE], F32, tag="db", bufs=2,
                               name=f"pd{b}{st}{kt}{h}")
                pd_tiles[b, st, h, kt] = pd
                bound = min(max((kt + 1) * 128 - st * STRIPE, 0), STRIPE)
                for g2 in range(STRIPE // 512):
                    lo, hi = g2 * 512, g2 * 512 + 512
                    # [lo, min(hi,bound)): qi < ki -> negated KA so the
                    # whole tile uses exp(+c * pd)
                    if bound > lo:
                        e = min(hi, bound)
                        nc.tensor.matmul(
                            pd[:, lo:e],
                            kan[b, h][:, kt * 128:(kt + 1) * 128],
                            qa[b, h][:, st * STRIPE + lo:st * STRIPE + e],
                            start=True, stop=True)
                    if bound < hi:
                        s0 = max(lo, bound)
                        nc.tensor.matmul(
                            pd[:, s0:hi],
                            ka[b, h][:, kt * 128:(kt + 1) * 128],
                            qa[b, h][:, st * STRIPE + s0:st * STRIPE + hi],
                            start=True, stop=True)
                if kt * 128 >= st * STRIPE and \
                   (kt + 1) * 128 <= (st + 1) * STRIPE:
                    dl = kt * 128 - st * STRIPE
                    nc.vector.tensor_tensor(
                        pd[:, dl:dl + 128], pd[:, dl:dl + 128],
                        sgnc, ALU.mult)

            def issue_exp(b, st, h, kt):
                pd = pd_tiles[b, st, h, kt]
                ptile = work.tile([128, STRIPE], BF16, tag="pt", bufs=6,
                                  name=f"pt{b}{st}{kt}{h}")
                nc.scalar.activation(ptile[:], pd[:], AF.Exp, scale=CEXP)
                return ptile

            units = [(b, st, h) for b in range(B) for st in range(NST)
                     for h in range(2)]

            def make_tail(b, st, h, po):
                def tail():
                    # normalize rows 0:64 by row 64 (softmax denominator)
                    den = work.tile([1, 2 * STRIPE], F32, tag="den",
                                    bufs=2, name=f"den{b}{st}{h}")
                    nc.vector.tensor_copy(den[:, 0:STRIPE], po[64:65, :])
                    nc.vector.reciprocal_approx_fast(
                        den[:, STRIPE:], den[:, 0:STRIPE])
                    bc_sb = work.tile([64, STRIPE], F32R, tag="bcsb",
                                      bufs=2, name=f"bc{b}{st}{h}")
                    for g2 in range(2):
                        pbc = psum.tile([64, 512], F32, tag="d", bufs=2,
                                        name=f"pbc{b}{st}{h}{g2}")
                        nc.tensor.matmul(
                            pbc[:], ones_row[:, 0:64],
                            den[:, STRIPE + g2 * 512:
                                STRIPE + (g2 + 1) * 512],
                            start=True, stop=True)
                        nc.vector.tensor_copy(
                            bc_sb[:, g2 * 512:(g2 + 1) * 512], pbc[:])
                    nc.vector.tensor_tensor(
                        outT[b][h * 64:(h + 1) * 64,
                                st * STRIPE:(st + 1) * STRIPE],
                        po[0:64, :], bc_sb[:], ALU.mult)
                    if h == 1:
                        # stripe complete: per-stripe AllToAll re-shards
                        # feature-blocks -> token-blocks for LayerNorm
                        q = b * NST + st
                        nc.sync.dma_start(
                            a2a_in[q][:].rearrange("(j p) n -> p j n", p=128),
                            outT[b][:, st * STRIPE:(st + 1) * STRIPE]
                            .rearrange("p (j n) -> p j n", n=HTOK))
                        nc.gpsimd.collective_compute(
                            "AllToAll", ALU.bypass,
                            replica_groups=[list(range(NCORES))],
                            ins=[a2a_in[q].opt()], outs=[a2a_out[q].opt()])
                        # layer_norm(q) is NOT issued here: its PE matmuls
                        # would head-of-line block the PE FIFO on the
                        # collective (observed 18us stall); deferred to
                        # ~2 units later in the main loop
                return tail

            # b0 projections (ScalarE is necessarily idle here; keep PE+DVE
            # dense). Part order q,v,k: v's matmuls reuse q's psum ring
            # slot only after q's 2-op epilogue (vs k's 6-op one), so the
            # PE never waits on the DVE queue between parts.
            xt_load(0, 0)
            xt_load(0, 1)
            load_consts()
            for qc in range(NQC):
                if qc + 2 < NQC:
                    xt_load(0, qc + 2)
                for which in ("q", "v", "k"):
                    proj_part(0, qc, which)

            # prefetch b1's first token group before attention starts
            xt_load(1, 0)

            # b1 projection work, fed into attention(b0)'s PE slack
            thunks = []
            for qc in range(NQC):
                for which in ("q", "v", "k"):
                    def mk(qc=qc, which=which):
                        def run():
                            if which == "q" and qc + 1 < NQC:
                                xt_load(1, qc + 1)
                            proj_part(1, qc, which)
                        return run
                    thunks.append(mk())
            thunks.reverse()  # pop() from the front

            prev_tail = None
            for ui, (b, st, h) in enumerate(units):
                issue_score(b, st, h, 0)
                issue_score(b, st, h, 1)
                if prev_tail is not None:
                    prev_tail()
                # deferred LayerNorms: far enough after their AllToAll
                # that the collective (rendezvous + transfer, ~70us for
                # the first one) is done before the PE reaches them
                if ui == 5:
                    layer_norm(0)
                elif ui == 7:
                    layer_norm(1)
                po = psum.tile([65, STRIPE], F32, tag="o",
                               name=f"po{h}_{b}{st}")
                for kt in range(NKT):
                    ptile = issue_exp(b, st, h, kt)
                    if kt + 2 < NKT:
                        issue_score(b, st, h, kt + 2)
                    for g2 in range(STRIPE // 512):
                        nc.tensor.matmul(
                            po[:, g2 * 512:(g2 + 1) * 512],
                            vsb[b, h][:, kt * 65:(kt + 1) * 65],
                            ptile[:, g2 * 512:(g2 + 1) * 512],
                            start=(kt == 0), stop=(kt == NKT - 1))
                    if kt in (3, 7, 11) and thunks:
                        thunks.pop()()
                pd_tiles.clear()
                prev_tail = make_tail(b, st, h, po)
            prev_tail()
            layer_norm(2)
            layer_norm(3)

    nc.compile()
    return nc


def _host_inputs(x, Wq, Wk, Wv, ln_gamma, ln_beta):
    import ml_dtypes
    bf16 = ml_dtypes.bfloat16
    xT = np.ascontiguousarray(
        x.reshape(NT, E).T.astype(np.float32)).astype(bf16)
    cst = np.zeros((128, S + 129), np.float32)
    cst[:, 0:S] = np.arange(S, dtype=np.float32)[None, :]
    jj = np.arange(128, dtype=np.float32)
    cst[:, S:S + 128] = -np.sign(jj[None, :] - jj[:, None])
    cst[:, S + 128] = 1.0
    gb = np.zeros((128, 2 * NET), np.float32)
    gb[:, 0:NET] = np.asarray(ln_gamma, np.float32).reshape(NET, 128).T
    gb[:, NET:] = np.asarray(ln_beta, np.float32).reshape(NET, 128).T
    in_maps = []
    for c in range(NCORES):
        fsl = slice(c * PF, (c + 1) * PF)
        in_maps.append({
            "xT": xT,
            "wq": np.ascontiguousarray(
                np.asarray(Wq, np.float32)[fsl, :].T).astype(bf16),
            "wk": np.ascontiguousarray(
                np.asarray(Wk, np.float32)[fsl, :].T).astype(bf16),
            "wv": np.ascontiguousarray(
                np.asarray(Wv, np.float32)[fsl, :].T).astype(bf16),
            "cst": cst,
            "onesr": np.ones((1, 128), np.float32),
            "identb": np.eye(128).astype(bf16),
            "gb": gb,
        })
    return in_maps


def kernel(x, Wq, Wk, Wv, ln_gamma, ln_beta, _trace=False, _tmpdir=None):
    ln_trivial = bool(np.all(np.asarray(ln_gamma) == 1.0)
                      and np.all(np.asarray(ln_beta) == 0.0))
    key = ("nc", ln_trivial)
    if key not in _CACHE:
        _CACHE[key] = _build(ln_trivial)
    nc = _CACHE[key]
    in_maps = _host_inputs(x, Wq, Wk, Wv, ln_gamma, ln_beta)
    res = bass_utils.run_bass_kernel_spmd(
        nc, in_maps, core_ids=list(range(NCORES)),
        trace=_trace, tmpdir=_tmpdir)
    _CACHE["last_result"] = res
    # out[c] is [E, 4*HTOK]: quarter q=(b*2+st) at cols [q*128,(q+1)*128)
    # = tokens [b*S + st*1024 + 128c, +128).
    outT = np.empty((E, NT), np.float32)
    for c in range(NCORES):
        o = np.asarray(res.results[c]["out"])
        for q in range(4):
            b, st = q // 2, q % 2
            base = b * S + st * STRIPE + c * HTOK
            outT[:, base:base + HTOK] = o[:, q * HTOK:(q + 1) * HTOK]
    return np.ascontiguousarray(outT.T).reshape(B, S, E).astype(np.float32)
